# revision 14
# baseline (speedup 1.0000x reference)
import os
import sys
from contextlib import ExitStack

import numpy as np

sys.path.insert(0, "/opt/trn_rl_repo")

from concourse import bacc, bass, mybir, tile  # noqa: E402
from concourse.bass_utils import run_bass_kernel_spmd  # noqa: E402
from concourse.masks import make_identity  # noqa: E402

# problem constants (hardcoded per spec)
B = 4
W_BEV = 512
H_BEV = 512
M = 200000
N = 2000000
BN_EPS = 1e-3
NDEV = 8

# kernel layout constants
G_COLS = 2048          # point-columns per work group
CHUNK = 512            # matmul/PSUM chunk (one PSUM bank of fp32)
SEG_SLOTS_MAX = 3328   # pillar-pair slots per segment (multiple of 128)


def _preprocess(inputs):
    gf = np.ascontiguousarray(np.asarray(inputs["group_features"], dtype=np.float32))
    psi = np.asarray(inputs["pillar_set_indices"], dtype=np.int64)
    pin = np.asarray(inputs["pillar_indices"], dtype=np.int64)
    W0 = np.asarray(inputs["W0"], dtype=np.float32)
    gamma = np.asarray(inputs["gamma"], dtype=np.float32)
    beta = np.asarray(inputs["beta"], dtype=np.float32)
    Ws = np.asarray(inputs["Ws"], dtype=np.float32)
    bs = np.asarray(inputs["bs"], dtype=np.float32)

    half_x = W_BEV // 2
    canvas_rows = half_x * H_BEV
    pad_cell = canvas_rows  # > bounds_check -> silently dropped by indirect DMA

    n_pts = gf.shape[0]
    order = np.argsort(psi, kind="stable").astype(np.int64)
    counts = np.bincount(psi, minlength=M).astype(np.int64)
    starts = np.zeros(M + 1, np.int64)
    np.cumsum(counts, out=starts[1:])

    mb = M // B
    pil_b = np.arange(M, dtype=np.int64) // mb
    x = pin[:, 1]
    y = pin[:, 2]
    xh = (x >= half_x).astype(np.int64)
    pil_dev = pil_b * 2 + xh
    cell_all = (x - half_x * xh) * H_BEV + y  # (M,) row in the device canvas

    kmax = int(counts.max())
    assert kmax <= G_COLS

    # per-device per-class pillar lists; shared slot counts (one slot = one
    # A-pillar + one B-pillar of the same class k, occupying k columns)
    slots_k = np.zeros(kmax + 1, np.int64)
    dev_class = []
    for d in range(NDEV):
        ids_d = np.nonzero(pil_dev == d)[0]
        cd = counts[ids_d]
        o2 = np.argsort(cd, kind="stable")
        ids_sorted = ids_d[o2]
        cs = cd[o2]
        lo = np.searchsorted(cs, np.arange(1, kmax + 2), side="left")
        per_k = [ids_sorted[lo[k - 1]:lo[k]] for k in range(1, kmax + 1)]
        dev_class.append(per_k)
        for k in range(1, kmax + 1):
            nk = len(per_k[k - 1])
            slots_k[k] = max(slots_k[k], (nk + 1) // 2)

    cs0 = np.zeros(kmax + 2, np.int64)
    np.cumsum(slots_k, out=cs0[1:])
    s_tot = int(cs0[-1])
    slot_class = np.repeat(np.arange(kmax + 1), slots_k)  # (s_tot,)
    slot_col0 = np.zeros(s_tot + 1, np.int64)
    np.cumsum(slot_class, out=slot_col0[1:])
    f_tot = int(slot_col0[-1])

    slotA = np.full((NDEV, s_tot), -1, np.int64)
    slotB = np.full((NDEV, s_tot), -1, np.int64)
    for k in range(1, kmax + 1):
        sk = int(slots_k[k])
        if sk == 0:
            continue
        s0 = int(cs0[k])
        for d in range(NDEV):
            ids = dev_class[d][k - 1]
            na = (len(ids) + 1) // 2
            slotA[d, s0:s0 + na] = ids[:na]
            slotB[d, s0:s0 + len(ids) - na] = ids[na:]

    # segments (slot ranges) and groups (column ranges, slot-aligned)
    n_seg = max(1, -(-s_tot // SEG_SLOTS_MAX))
    sb = [round(s_tot * i / n_seg) for i in range(n_seg + 1)]
    segs = []
    blk_off = 0
    for si in range(n_seg):
        s0, s1 = sb[si], sb[si + 1]
        ns = s1 - s0
        p128 = -(-ns // 128) * 128
        nblk2 = 2 * (p128 // 128)
        groups = []
        s = s0
        while s < s1:
            gcol0 = int(slot_col0[s])
            e = s
            while e < s1 and int(slot_col0[e + 1]) - gcol0 <= G_COLS:
                e += 1
            assert e > s
            pieces = []
            r = s
            while r < e:
                k = int(slot_class[r])
                r2 = min(e, int(cs0[k + 1]))
                pieces.append((int(slot_col0[r]) - gcol0, r2 - r, k, r - s0))
                r = r2
            groups.append((gcol0, int(slot_col0[e]), pieces))
            s = e
        segs.append(dict(slot0=s0, slot1=s1, ns=ns, p128=p128, nblk2=nblk2,
                         blk_off=blk_off, groups=groups))
        blk_off += nblk2
    nblk_tot = blk_off

    # exact BN batch stats (f64) folded into the first matmul:
    # h_pre = X @ W0; mu = mean(h_pre); var = E[h_pre^2] - mu^2
    # a = gamma * rsqrt(var+eps); b = beta - mu*a
    # W0' = [[W0 * a], [b]] applied to X' = [x; 1]
    gf64 = gf.astype(np.float64)
    S = gf64.T @ gf64
    sx = gf64.sum(axis=0)
    W64 = W0.astype(np.float64)
    mu = (sx @ W64) / n_pts
    e2 = np.einsum("ic,ij,jc->c", W64, S, W64) / n_pts
    var = e2 - mu * mu
    a = gamma.astype(np.float64) / np.sqrt(var + BN_EPS)
    bfold = beta.astype(np.float64) - mu * a
    w0f = np.zeros((11, 64), np.float64)
    w0f[:10] = W64 * a[None, :]
    w0f[10] = bfold

    w0p_arr = np.zeros((128, 64), np.float32)
    w0p_arr[0:11] = w0f.astype(np.float32)
    w0p_arr[64:75] = w0f.astype(np.float32)
    ws_arr = np.zeros((128, 64), np.float32)
    ws_arr[0:64] = Ws
    ws_arr[64:128] = Ws
    bs_arr = np.zeros((128, 1), np.float32)
    bs_arr[0:64, 0] = bs
    bs_arr[64:128, 0] = bs

    # per-device xt (22, f_tot): rows 0-9 A-point feats, 10 A-ones,
    # 11-20 B-point feats, 21 B-ones; pads are all-zero columns
    xts = []
    idxs = []
    cellA = np.where(slotA >= 0, cell_all[np.maximum(slotA, 0)], pad_cell).astype(np.int32)
    cellB = np.where(slotB >= 0, cell_all[np.maximum(slotB, 0)], pad_cell).astype(np.int32)
    for d in range(NDEV):
        ptA = np.full(f_tot, -1, np.int64)
        ptB = np.full(f_tot, -1, np.int64)
        for k in range(1, kmax + 1):
            sk = int(slots_k[k])
            if sk == 0:
                continue
            s0 = int(cs0[k])
            col0s = int(slot_col0[s0]) + k * np.arange(sk, dtype=np.int64)
            for slot_arr, pt_arr in ((slotA, ptA), (slotB, ptB)):
                pids = slot_arr[d, s0:s0 + sk]
                real = pids >= 0
                if not np.any(real):
                    continue
                pr = pids[real]
                gidx = starts[pr][:, None] + np.arange(k)[None, :]
                pts = order[gidx]
                cols = col0s[real][:, None] + np.arange(k)[None, :]
                pt_arr[cols.ravel()] = pts.ravel()
        xt = np.zeros((22, f_tot), np.float32)
        mA = ptA >= 0
        xt[0:10, mA] = gf[ptA[mA]].T
        xt[10, mA] = 1.0
        mB = ptB >= 0
        xt[11:21, mB] = gf[ptB[mB]].T
        xt[21, mB] = 1.0
        xts.append(np.ascontiguousarray(xt))

        # idx layout: per segment, column 2c+half holds cells of slots
        # [slot0 + c*128, slot0 + (c+1)*128) for half A(0)/B(1)
        idx_d = np.full((128, nblk_tot), pad_cell, np.int32)
        for seg in segs:
            ns, p128 = seg["ns"], seg["p128"]
            nb = p128 // 128
            bo = seg["blk_off"]
            bufA = np.full(p128, pad_cell, np.int32)
            bufA[:ns] = cellA[d, seg["slot0"]:seg["slot1"]]
            bufB = np.full(p128, pad_cell, np.int32)
            bufB[:ns] = cellB[d, seg["slot0"]:seg["slot1"]]
            idx_d[:, bo + 0:bo + 2 * nb:2] = bufA.reshape(nb, 128).T
            idx_d[:, bo + 1:bo + 2 * nb:2] = bufB.reshape(nb, 128).T
        idxs.append(np.ascontiguousarray(idx_d))

    return dict(
        f_tot=f_tot, nblk_tot=nblk_tot, segs=segs, s_tot=s_tot,
        slot_class=slot_class, slot_col0=slot_col0, cs0=cs0, slots_k=slots_k,
        slotA=slotA, slotB=slotB, cellA=cellA, cellB=cellB, kmax=kmax,
        xts=xts, idxs=idxs, w0p=w0p_arr, ws=ws_arr, bsv=bs_arr, w0f=w0f,
        canvas_rows=canvas_rows, pad_cell=pad_cell,
    )


def _build_nc(pre):
    f32 = mybir.dt.float32
    i32 = mybir.dt.int32
    AF = mybir.ActivationFunctionType
    AL = mybir.AluOpType
    AX = mybir.AxisListType

    nc = bacc.Bacc("TRN2", debug=False, target_bir_lowering=False)
    xt_d = nc.dram_tensor("xt", (22, pre["f_tot"]), f32, kind="ExternalInput").ap()
    idx_d = nc.dram_tensor("idx", (128, pre["nblk_tot"]), i32, kind="ExternalInput").ap()
    w0p_d = nc.dram_tensor("w0p", (128, 64), f32, kind="ExternalInput").ap()
    ws_d = nc.dram_tensor("ws", (128, 64), f32, kind="ExternalInput").ap()
    bsv_d = nc.dram_tensor("bsv", (128, 1), f32, kind="ExternalInput").ap()
    canvas_d = nc.dram_tensor("canvas", (pre["canvas_rows"], 64), f32,
                              kind="ExternalOutput").ap()

    seg_slots_max = max(seg["p128"] for seg in pre["segs"])
    nblk2_max = max(seg["nblk2"] for seg in pre["segs"])

    with tile.TileContext(nc) as tc, ExitStack() as ctx:
        const_tp = ctx.enter_context(tc.tile_pool(name="const", bufs=1))
        xt_tp = ctx.enter_context(tc.tile_pool(name="xtp", bufs=2))
        h_tp = ctx.enter_context(tc.tile_pool(name="hp", bufs=2))
        e_tp = ctx.enter_context(tc.tile_pool(name="ep", bufs=2))
        ec_tp = ctx.enter_context(tc.tile_pool(name="ecp", bufs=2))
        esum_tp = ctx.enter_context(tc.tile_pool(name="esp", bufs=1))
        hesum_tp = ctx.enter_context(tc.tile_pool(name="hesp", bufs=1))
        hmax_tp = ctx.enter_context(tc.tile_pool(name="hmp", bufs=1))
        ra_tp = ctx.enter_context(tc.tile_pool(name="rap", bufs=1))
        rb_tp = ctx.enter_context(tc.tile_pool(name="rbp", bufs=1))
        pm_tp = ctx.enter_context(tc.tile_pool(name="pmp", bufs=2))
        ps1_tp = ctx.enter_context(tc.tile_pool(name="ps1p", bufs=2, space="PSUM"))
        ps2_tp = ctx.enter_context(tc.tile_pool(name="ps2p", bufs=2, space="PSUM"))
        pt_tp = ctx.enter_context(tc.tile_pool(name="ptp", bufs=2, space="PSUM"))

        w0p_sb = const_tp.tile([128, 64], f32, name="w0p_sb")
        ws_sb = const_tp.tile([128, 64], f32, name="ws_sb")
        bsv_sb = const_tp.tile([128, 1], f32, name="bsv_sb")
        idx_sb = const_tp.tile([128, pre["nblk_tot"]], i32, name="idx_sb")
        ident = const_tp.tile([128, 128], f32, name="ident")
        nc.sync.dma_start(w0p_sb, w0p_d)
        nc.sync.dma_start(ws_sb, ws_d)
        nc.sync.dma_start(bsv_sb, bsv_d)
        nc.sync.dma_start(idx_sb, idx_d)
        make_identity(nc, ident)

        stage = int(os.environ.get("KSTAGE", "6"))
        for seg in pre["segs"]:
            ns, p128, nblk2 = seg["ns"], seg["p128"], seg["nblk2"]
            esum = esum_tp.tile([128, seg_slots_max], f32, name="esum")
            hesum = hesum_tp.tile([128, seg_slots_max], f32, name="hesum")
            hmax = hmax_tp.tile([128, seg_slots_max], f32, name="hmax")
            if p128 > ns:
                nc.gpsimd.memset(esum[:, ns:p128], 1.0)
                nc.gpsimd.memset(hesum[:, ns:p128], 0.0)
                nc.gpsimd.memset(hmax[:, ns:p128], 0.0)
            for (gc0, gc1, pieces) in seg["groups"]:
                if stage < 2:
                    break
                gc = gc1 - gc0
                xt_t = xt_tp.tile([128, G_COLS], f32, name="xt_t")
                h_t = h_tp.tile([128, G_COLS], f32, name="h_t")
                e_t = e_tp.tile([128, G_COLS], f32, name="e_t")
                ec_t = ec_tp.tile([128, G_COLS], f32, name="ec_t")
                nc.sync.dma_start(xt_t[0:11, 0:gc], xt_d[0:11, gc0:gc1])
                nc.sync.dma_start(xt_t[64:75, 0:gc], xt_d[11:22, gc0:gc1])
                for c0 in range(0, gc, CHUNK):
                    cw = min(CHUNK, gc - c0)
                    ps1 = ps1_tp.tile([128, CHUNK], f32, name="ps1")
                    ps2 = ps2_tp.tile([128, CHUNK], f32, name="ps2")
                    nc.tensor.matmul(ps1[0:64, 0:cw], lhsT=w0p_sb[0:11, 0:64],
                                     rhs=xt_t[0:11, c0:c0 + cw], start=True, stop=True)
                    nc.tensor.matmul(ps1[64:128, 0:cw], lhsT=w0p_sb[64:75, 0:64],
                                     rhs=xt_t[64:75, c0:c0 + cw], start=True, stop=True)
                    nc.scalar.activation(h_t[:, c0:c0 + cw], ps1[:, 0:cw], AF.Relu)
                    nc.tensor.matmul(ps2[0:64, 0:cw], lhsT=ws_sb[0:64, 0:64],
                                     rhs=h_t[0:64, c0:c0 + cw], start=True, stop=True)
                    nc.tensor.matmul(ps2[64:128, 0:cw], lhsT=ws_sb[64:128, 0:64],
                                     rhs=h_t[64:128, c0:c0 + cw], start=True, stop=True)
                    nc.scalar.activation(e_t[:, c0:c0 + cw], ps2[:, 0:cw], AF.Exp,
                                         bias=bsv_sb[:, 0:1])
                # ec = max(e, 1) == exp(relu(z+bs)); then e_t <- h*ec
                if stage < 3:
                    continue
                nc.vector.tensor_scalar_max(ec_t[:, 0:gc], e_t[:, 0:gc], 1.0)
                nc.vector.tensor_tensor(e_t[:, 0:gc], h_t[:, 0:gc], ec_t[:, 0:gc],
                                        op=AL.mult)
                for (o, m, k, q) in pieces:
                    nc.vector.tensor_reduce(
                        esum[:, q:q + m],
                        ec_t[:, o:o + m * k].rearrange("p (m k) -> p m k", k=k),
                        axis=AX.X, op=AL.add)
                    nc.vector.tensor_reduce(
                        hesum[:, q:q + m],
                        e_t[:, o:o + m * k].rearrange("p (m k) -> p m k", k=k),
                        axis=AX.X, op=AL.add)
                    nc.vector.tensor_reduce(
                        hmax[:, q:q + m],
                        h_t[:, o:o + m * k].rearrange("p (m k) -> p m k", k=k),
                        axis=AX.X, op=AL.max)
            # pillar features: p = 0.5*hmax + 0.5*hesum/esum
            if stage < 4:
                continue
            ra = ra_tp.tile([128, seg_slots_max], f32, name="ra")
            rb = rb_tp.tile([128, seg_slots_max], f32, name="rb")
            nc.vector.reciprocal(ra[:, 0:p128], esum[:, 0:p128])
            nc.vector.scalar_tensor_tensor(rb[:, 0:p128], ra[:, 0:p128], 0.5,
                                           hesum[:, 0:p128], op0=AL.mult, op1=AL.mult)
            nc.vector.scalar_tensor_tensor(ra[:, 0:p128], hmax[:, 0:p128], 0.5,
                                           rb[:, 0:p128], op0=AL.mult, op1=AL.add)
            # transpose to pillar-major rows and scatter into the canvas
            if stage < 5:
                continue
            nblk = p128 // 128
            p_pm = pm_tp.tile([128, nblk2_max * 64], f32, name="p_pm")
            j = 0
            while j < nblk:
                jn = min(max(CHUNK // 128, 1), nblk - j)
                ptt = pt_tp.tile([128, max(CHUNK, 128)], f32, name="ptt")
                for u in range(jn):
                    c = j + u
                    nc.tensor.transpose(ptt[:, u * 128:(u + 1) * 128],
                                        ra[:, c * 128:(c + 1) * 128],
                                        ident[:, 0:128])
                nc.scalar.activation(p_pm[:, j * 128:(j + jn) * 128],
                                     ptt[:, 0:jn * 128], AF.Copy)
                j += jn
            if stage < 6:
                continue
            for blk in range(nblk2):
                c, half = divmod(blk, 2)
                nc.gpsimd.indirect_dma_start(
                    out=canvas_d,
                    out_offset=bass.IndirectOffsetOnAxis(
                        ap=idx_sb[:, seg["blk_off"] + blk:seg["blk_off"] + blk + 1],
                        axis=0),
                    in_=p_pm[:, c * 128 + half * 64:c * 128 + (half + 1) * 64],
                    in_offset=None,
                    bounds_check=pre["canvas_rows"] - 1,
                    oob_is_err=False)
    nc.finalize()
    return nc


def _in_maps(pre):
    return [dict(xt=pre["xts"][d], idx=pre["idxs"][d], w0p=pre["w0p"],
                 ws=pre["ws"], bsv=pre["bsv"]) for d in range(NDEV)]


def _assemble(canvases):
    half_x = W_BEV // 2
    out = np.empty((B, 64, W_BEV, H_BEV), np.float32)
    for d in range(NDEV):
        cv = np.asarray(canvases[d]).reshape(half_x, H_BEV, 64)
        b, xh = divmod(d, 2)
        out[b, :, xh * half_x:(xh + 1) * half_x, :] = cv.transpose(2, 0, 1)
    return out


def _install_trace_hooks():
    import types

    try:
        import antenv.axon_hooks as ah
    except Exception:
        ah = types.ModuleType("antenv.axon_hooks")
        _state = {"hook": None}
        ah.set_axon_ntff_profile_hook = lambda h: _state.__setitem__("hook", h)
        ah.get_axon_ntff_profile_hook = lambda: _state["hook"]
        sys.modules["antenv.axon_hooks"] = ah
    try:
        if ah.get_axon_ntff_profile_hook() is None:
            if "/root/.axon_site" not in sys.path:
                sys.path.insert(0, "/root/.axon_site")
            from trn_agent_boot.trn_boot import _ntff_profile_via_ctypes

            hook = _ntff_profile_via_ctypes("/opt/axon/libaxon_pjrt.so")
            if hook is not None:
                ah.set_axon_ntff_profile_hook(hook)
    except Exception:
        pass
    try:
        import concourse.bass_utils as bu

        bu.upload_artifacts = lambda tmpdir: "file://" + tmpdir
    except Exception:
        pass


def kernel(**inputs):
    pre = _preprocess(inputs)
    nc = _build_nc(pre)
    trace = os.environ.get("KERNEL_TRACE", "") == "1"
    if trace:
        _install_trace_hooks()
    res = run_bass_kernel_spmd(nc, _in_maps(pre), list(range(NDEV)), trace=trace)
    kernel.last_results = res
    return _assemble([res.results[d]["canvas"] for d in range(NDEV)])


kernel.last_results = None
